# revision 24
# baseline (speedup 1.0000x reference)
"""GQA attention forward (B=4, T=1024, D=2048, 32 q-heads / 8 kv-heads, RoPE,
causal) distributed over 8 TRN2 NeuronCores.

Sharding: head-parallel tensor parallelism. Core c owns q-heads 4c..4c+3 and
kv-head c (wq/wk/wv column shards). Attention output (still sharded by head,
transposed layout [head_dim, tokens]) is re-sharded to token-parallel via one
AllToAll (512KB/rank/batch, bf16); each core then computes its 512-token row
slice of the output projection against the full wo.

Device layouts (per core):
  xT    [2048, 4096]  bf16  - x transposed, tokens batch-major
  qT    [128, 4096]x2 bf16  - 2 heads per tile, RoPE'd; head-dim de-interleaved
  kT2   [128, 4096]   bf16  - kv-head kT duplicated in both 64-partition halves
  vT    [64, 4096]    bf16  - PE-transposed per 128-token tile into
                              v[128, 66] with ones columns 64,65 (softmax
                              denominator lands in po rows 64/65 per h01)
  scores sT[k, q] in PSUM -> exp on ACT (scale=1/8 folded) -> bf16
  causal diag mask via gpsimd affine_select (q - k >= 0)
  attn@v: lhsT = v_aug [128, 65+h01], rhs = expT -> psum [65+h01, 512]
  softmax divide: den rows DVE-copied (32-aligned shuffle) into a small pack,
  reciprocal_approx_fast (2 lanes/hp), gpsimd partition_broadcast, fused
  (po x rec) multiply straight into per-head aoT2 tiles
  A2A -> aoT_g [2048(c), 512(t)] -> out[t, e] = sum_c aoT_g[c, t] * wo[c, e]

RoPE with de-interleaved head dims ([32 reals; 32 imags] per 64-row head):
  out = x*C + shift32(x*S), C = [c;c;...], S = [s;-s;s;-s] (host-built tiles);
  the 32-row shift copies run on gpsimd to keep DVE free.
"""

import sys

if "/opt/trn_rl_repo" not in sys.path:
    sys.path.insert(0, "/opt/trn_rl_repo")

import numpy as np
import ml_dtypes

import concourse.bass as bass
import concourse.mybir as mybir
import concourse.tile as tile
from concourse import bacc
from concourse.bass_utils import run_bass_kernel_spmd
from concourse.masks import make_identity

BF16 = mybir.dt.bfloat16
F32 = mybir.dt.float32

B, T, D = 4, 1024, 2048
QH, KVH, HD = 32, 8, 64
N_CORES = 8
NT = B * T            # 4096 global tokens
NKO = D // 128        # 16 contraction subtiles
ROWS = NT // N_CORES  # 512 output rows per core
HPC = QH // N_CORES   # 4 q heads per core

_CACHE = {}


def _enable_ldw_opt():
    # walrus ships with --enable-ldw-opt=false hardcoded; LDWEIGHTS then
    # serializes with matmul streams. Flipping it on FAILS codegen in this
    # walrus build ("InstLdweights is not compatible with LDW optimization"
    # even for plain [128,128] bf16 weights), so it stays opt-in via env.
    import os
    if not os.environ.get("FORCE_LDW_ON"):
        return
    import concourse.bass_utils as _bu
    if getattr(_bu, "_ldw_patched", False):
        return
    _orig = _bu.run_command

    def _patched(argv, **kw):
        argv = [a.replace("--enable-ldw-opt=false", "--enable-ldw-opt=true")
                if isinstance(a, str) else a for a in argv]
        return _orig(argv, **kw)

    _bu.run_command = _patched
    _bu._ldw_patched = True


def _build():
    _enable_ldw_opt()
    nc = bacc.Bacc("TRN2", target_bir_lowering=False, debug=False,
                   num_devices=N_CORES)

    xT = nc.dram_tensor("xT", [8, 128, NKO, 512], BF16, kind="ExternalInput")
    wq = nc.dram_tensor("wq", [128, NKO, HPC * HD], BF16,
                        kind="ExternalInput")
    wkv = nc.dram_tensor("wkv", [128, NKO, 2 * HD], BF16,
                         kind="ExternalInput")
    wo = nc.dram_tensor("wo", [128, NKO, D], BF16, kind="ExternalInput")
    ct = nc.dram_tensor("ctile", [128, T], BF16, kind="ExternalInput")
    st = nc.dram_tensor("stile", [128, T], BF16, kind="ExternalInput")
    out = nc.dram_tensor("out", [ROWS, D], F32, kind="ExternalOutput")

    xT_r = xT.ap()
    wq_r = wq.ap()
    wkv_r = wkv.ap()
    wo_r = wo.ap()

    import contextlib
    with tile.TileContext(nc) as tc, contextlib.ExitStack() as ctx:
        const = ctx.enter_context(tc.tile_pool(name="const", bufs=1))
        xp = ctx.enter_context(tc.tile_pool(name="xp", bufs=2))
        big = ctx.enter_context(tc.tile_pool(name="big", bufs=1))
        vp = ctx.enter_context(tc.tile_pool(name="vp", bufs=2))
        ep = ctx.enter_context(tc.tile_pool(name="ep", bufs=3))
        xsp = ctx.enter_context(tc.tile_pool(name="xsp", bufs=2))
        dnp = ctx.enter_context(tc.tile_pool(name="dnp", bufs=2))
        bcp = ctx.enter_context(tc.tile_pool(name="bcp", bufs=2))
        gp = ctx.enter_context(tc.tile_pool(name="gp", bufs=1))
        op = ctx.enter_context(tc.tile_pool(name="op", bufs=2))
        dram = ctx.enter_context(tc.tile_pool(name="dram", bufs=1,
                                              space="DRAM"))
        pp = ctx.enter_context(tc.tile_pool(name="pp", bufs=2, space="PSUM"))
        sp = ctx.enter_context(tc.tile_pool(name="sp", bufs=2, space="PSUM"))
        ap = ctx.enter_context(tc.tile_pool(name="ap", bufs=2, space="PSUM"))

        # constants / weights (spread across DMA queues so batch-0 x loads
        # aren't stuck behind them on the sync ring)
        wq_sb = const.tile([128, NKO, HPC * HD], BF16, tag="wq")
        nc.scalar.dma_start(wq_sb[:], wq_r)
        wkv_sb = const.tile([128, NKO, 2 * HD], BF16, tag="wkv")
        nc.scalar.dma_start(wkv_sb[:], wkv_r)
        ct_sb = const.tile([128, T], BF16, tag="ct")
        nc.scalar.dma_start(ct_sb[:], ct.ap())
        st_sb = const.tile([128, T], BF16, tag="st")
        nc.scalar.dma_start(st_sb[:], st.ap())
        ident = const.tile([64, 64], BF16, tag="ident")
        make_identity(nc, ident[:])
        # K=1 zero operands: a dummy matmul made from these zeroes a psum
        # bank AND sets every has_written bit, so col-split accumulation
        # below is order-independent
        zrow = const.tile([1, 128], BF16, tag="zrow")
        nc.any.memset(zrow[:], 0.0)
        rrow = const.tile([1, 512], BF16, tag="rrow")
        nc.any.memset(rrow[:], 0.0)

        qT = [big.tile([128, NT], BF16, tag=f"qT{hp}", name=f"qT{hp}")
              for hp in range(2)]
        kT2 = big.tile([128, NT], BF16, tag="kT2")
        # per-head attention output tiles: [64, NT] each so the softmax
        # divide writes land partition-aligned with psum rows 0..63
        aoT2 = [[big.tile([64, NT], BF16, tag=f"aoT{hp}{h01}",
                          name=f"aoT{hp}{h01}") for h01 in range(2)]
                for hp in range(2)]

        in_b = [dram.tile([N_CORES, 256, 128], BF16, tag=f"a2a_in{bb}",
                          name=f"a2a_in{bb}") for bb in range(B)]
        out_b = [dram.tile([N_CORES, 256, 128], BF16, tag=f"a2a_out{bb}",
                           name=f"a2a_out{bb}") for bb in range(B)]
        # full wo stays resident; streamed in during batch-0 compute
        wo_sb = const.tile([128, NKO, D], BF16, tag="wo")
        for n in range(4):
            nc.gpsimd.dma_start(wo_sb[:, :, n * 512:n * 512 + 512],
                                wo_r[:, :, n * 512:n * 512 + 512])

        import os as _os
        _ksplit_on = bool(_os.environ.get("KSPLIT"))

        def ksplit_mms(ps, lhs_of, rhs_of, n_ko):
            # contraction over n_ko K=128 subtiles. Col-split variant: each
            # K=128 matmul becomes two concurrent M=64 matmuls in disjoint
            # PE column groups writing disjoint partition halves of the
            # bank (no per-element write overlap). Only the very first
            # matmul carries start=True (bank-wide has_written clear at its
            # issue); the other column group's ko=0 matmul overwrites its
            # half where bits are unset.
            if not _ksplit_on:
                for ko in range(n_ko):
                    nc.tensor.matmul(
                        ps[0:128, :], lhs_of(ko, 0, 128), rhs_of(ko),
                        start=(ko == 0), stop=(ko == n_ko - 1))
                return
            nc.tensor.matmul(ps[0:128, :], zrow[:], rrow[:],
                             start=True, stop=False, skip_group_check=True)
            for ko in range(n_ko):
                for h in range(2):
                    nc.tensor.matmul(
                        ps[h * 64:h * 64 + 64, :],
                        lhs_of(ko, h * 64, 64), rhs_of(ko),
                        start=False,
                        stop=(ko == n_ko - 1 and h == 1),
                        skip_group_check=True)

        def wo_block(bb):
            # gather the A2A output of batch bb and project my 128 tokens
            aog_b = gp.tile([128, NKO, 128], BF16, tag="aog", bufs=2,
                            name="aog_b")
            nc.scalar.dma_start(
                aog_b[:], out_b[bb].rearrange("s (h p) q -> p (s h) q",
                                              p=128))
            for p in range(2):
                psh = [pp.tile([128, 512], F32, tag="mm", name=f"wops{q}")
                       for q in range(2)]
                for kk in range(2 * N_CORES):
                    for q in range(2):
                        n = 2 * p + q
                        nc.tensor.matmul(
                            psh[q][:], aog_b[:, kk, :],
                            wo_sb[:, kk, n * 512:n * 512 + 512],
                            start=(kk == 0), stop=(kk == 2 * N_CORES - 1),
                            skip_group_check=True)
                for q in range(2):
                    n = 2 * p + q
                    ot = op.tile([128, 512], F32, tag="ot", bufs=2)
                    nc.vector.tensor_copy(ot[:], psh[q][:])
                    nc.scalar.dma_start(
                        out.ap()[bb * 128:bb * 128 + 128,
                                 n * 512:n * 512 + 512], ot[:])

        def rope(dst, xs, xs2, ps, rows, cs_sl, ss_sl):
            # dst = ps*C + shift32(ps*S) over `rows` partitions (64 or 128)
            nc.vector.scalar_tensor_tensor(
                dst, ps[0:rows], 1.0, cs_sl[0:rows],
                mybir.AluOpType.mult, mybir.AluOpType.mult)
            nc.vector.scalar_tensor_tensor(
                xs[0:rows], ps[0:rows], 1.0, ss_sl[0:rows],
                mybir.AluOpType.mult, mybir.AluOpType.mult)
            # shift-by-32 within each 64-row half (cross-partition copies)
            for g in range(rows // 32):
                a, b_ = g * 32, (g ^ 1) * 32
                nc.vector.tensor_copy(xs2[a:a + 32], xs[b_:b_ + 32])
            nc.vector.tensor_add(dst, dst, xs2[0:rows])

        for b in range(B):
            xc = []
            for half in range(2):
                xt = xp.tile([128, NKO, 512], BF16, tag="x")
                eng = nc.sync if half == 0 else nc.scalar
                for kq in range(4):
                    eng.dma_start(
                        xt[:, kq * 4:kq * 4 + 4, :],
                        xT_r[b * 2 + half, :, kq * 4:kq * 4 + 4])
                xc.append(xt)

            # q projection + rope; both 512-token halves advance per ko so
            # the wq stationary tile is loaded once per matmul pair
            for hp in range(2):
                psh = [pp.tile([128, 512], F32, tag="mm", name=f"qps{half}")
                       for half in range(2)]
                for ko in range(NKO):
                    for half in range(2):
                        nc.tensor.matmul(
                            psh[half][:],
                            wq_sb[:, ko, hp * 128:hp * 128 + 128],
                            xc[half][:, ko, :],
                            start=(ko == 0), stop=(ko == NKO - 1),
                            skip_group_check=True)
                for half in range(2):
                    t0 = b * T + half * 512
                    xs = xsp.tile([128, 512], BF16, tag="xs")
                    xs2 = xsp.tile([128, 512], BF16, tag="xs2")
                    csl = ct_sb[:, half * 512:half * 512 + 512]
                    ssl = st_sb[:, half * 512:half * 512 + 512]
                    rope(qT[hp][:, t0:t0 + 512], xs, xs2, psh[half], 128,
                         csl, ssl)

            # kv projection: both halves advance per ko (shared wkv
            # stationary); rope k into both halves of kT2
            vstages = []
            kvps = [pp.tile([128, 512], F32, tag="mm", name=f"kvps{half}")
                    for half in range(2)]
            for ko in range(NKO):
                for half in range(2):
                    nc.tensor.matmul(
                        kvps[half][:], wkv_sb[:, ko, :], xc[half][:, ko, :],
                        start=(ko == 0), stop=(ko == NKO - 1),
                        skip_group_check=True)
            for half in range(2):
                t0 = b * T + half * 512
                ps = kvps[half]
                xs = xsp.tile([128, 512], BF16, tag="xs")
                xs2 = xsp.tile([128, 512], BF16, tag="xs2")
                csl = ct_sb[:, half * 512:half * 512 + 512]
                ssl = st_sb[:, half * 512:half * 512 + 512]
                rope(kT2[0:64, t0:t0 + 512], xs, xs2, ps, 64, csl, ssl)
                nc.vector.tensor_copy(kT2[64:128, t0:t0 + 512],
                                      kT2[0:64, t0:t0 + 512])
                vstage = xsp.tile([64, 512], BF16, tag="vstage",
                                  name=f"vstage{half}")
                nc.vector.tensor_copy(vstage[:], ps[64:128, :])
                vstages.append(vstage)

            # v into natural layout [tok, 64] + a ones column (softmax
            # denominator trick: den lands in po row 64)
            v_b = vp.tile([128, 8, 66], BF16, tag="v")
            for j in range(8):
                tps = pp.tile([128, 64], BF16, tag="mm")
                nc.tensor.transpose(
                    tps[:],
                    vstages[j // 4][:, (j % 4) * 128:(j % 4) * 128 + 128],
                    ident[:])
                nc.vector.tensor_copy(v_b[:, j, 0:HD], tps[:])
            nc.any.memset(v_b[:, :, HD:66], 1.0)

            # attention: all 4 heads advance together per (chunk, k-tile) so
            # the PE stream stays dense; K=64 score matmuls for h01=0/1 sit
            # in disjoint row groups and overlap in the array
            in_r = in_b[b].rearrange("j r q -> r j q")
            for c in range(2):
                for hp in range(2):
                    po2 = [ap.tile([65, 512], F32, tag="attn",
                                   name=f"po{h01}") for h01 in range(2)]
                    nj = 4 * c + 4
                    for j in range(nj):
                        q_lo = max(c * 512, j * 128)
                        N = (c + 1) * 512 - q_lo
                        q_rel = q_lo - c * 512
                        ks = b * T + j * 128
                        # both heads' scores into one 2-bank psum tile so
                        # exp runs once over [128, 2, N]
                        ps = sp.tile([128, 2, 512], F32, tag="score")
                        et = ep.tile([128, 2, 512], BF16, tag="et")
                        for h01 in range(2):
                            hbase = h01 * 64
                            nc.tensor.matmul(
                                ps[:, h01, :N],
                                kT2[hbase:hbase + 64, ks:ks + 128],
                                qT[hp][hbase:hbase + 64,
                                       b * T + q_lo:b * T + q_lo + N],
                                start=True, stop=True)
                        nc.scalar.activation(
                            et[:, :, :N], ps[:, :, :N],
                            mybir.ActivationFunctionType.Exp, scale=0.125)
                        if j >= 4 * c:
                            # causal diagonal block: keep q >= k, zero rest
                            # (2D per h01, the shape affine_select is proven
                            # on in masks.py)
                            for h01 in range(2):
                                nc.gpsimd.affine_select(
                                    out=et[:, h01, 0:128],
                                    in_=et[:, h01, 0:128],
                                    compare_op=mybir.AluOpType.is_ge,
                                    fill=0.0, base=0,
                                    pattern=[[1, 128]],
                                    channel_multiplier=-1)
                        for h01 in range(2):
                            nc.tensor.matmul(
                                po2[h01][0:65, q_rel:512],
                                v_b[:, j, 0:65],
                                et[:, h01, :N],
                                start=(j == 0), stop=(j == nj - 1))
                    # softmax normalization for this hp's 2 heads: fast
                    # reciprocal straight off the psum den row (aligned
                    # 64->0 shuffle), broadcast, then a fused (po * rec)
                    # multiply into the per-head aoT2 tile
                    g0 = (b * 2 + c) * 512
                    for h01 in range(2):
                        den1 = dnp.tile([1, 512], F32, tag="den", bufs=2)
                        nc.scalar.copy(den1[:], po2[h01][64:65, :])
                        rec1 = dnp.tile([1, 512], F32, tag="rec", bufs=2)
                        nc.vector.reciprocal_approx_fast(rec1[:], den1[:])
                        bc = bcp.tile([64, 512], F32, tag="bc", bufs=2)
                        nc.gpsimd.partition_broadcast(bc[:], rec1[:])
                        nc.vector.tensor_mul(
                            aoT2[hp][h01][:, g0:g0 + 512],
                            po2[h01][0:64, :], bc[:])
                    # stream this chunk's A2A input blocks out once divided:
                    # batch-b slot j carries my rows for token block b*8+j
                    for h01 in range(2):
                        nc.sync.dma_start(
                            in_r[hp * 128 + h01 * 64:
                                 hp * 128 + h01 * 64 + 64,
                                 c * 4:c * 4 + 4, :],
                            aoT2[hp][h01][:, g0:g0 + 512].rearrange(
                                "p (j q) -> p j q", j=4))

            # fire this batch's re-shard; its consumption (gather + wo) is
            # deferred by one batch so nothing ever waits on an in-flight
            # collective
            nc.gpsimd.collective_compute(
                "AllToAll", mybir.AluOpType.bypass,
                replica_groups=[list(range(N_CORES))],
                ins=[in_b[b].opt()], outs=[out_b[b].opt()])

            # consume A2As two batches late so the in-order PE stream never
            # parks on an in-flight collective (a full batch of compute
            # covers even a skew-delayed collective)
            if b >= 2:
                wo_block(b - 2)

        wo_block(B - 2)
        wo_block(B - 1)

    nc.compile()
    return nc


def _tile_k(w):
    # [D, M] -> [128, D//128, M] with d = ko*128 + p, contiguous per partition
    return np.ascontiguousarray(
        w.reshape(NKO, 128, w.shape[1]).transpose(1, 0, 2))


def _prep_inputs(x, wq, wk, wv, wo, cos, sin):
    bf = ml_dtypes.bfloat16
    x2 = x.reshape(NT, D).T  # [D, NT]
    # [8 chunks, 128, NKO, 512]: one contiguous 16KB run per partition
    xt = np.ascontiguousarray(
        x2.reshape(NKO, 128, 8, 512).transpose(2, 1, 0, 3)).astype(bf)
    # de-interleave rope pairs within each head: col j -> (j%2)*32 + j//2
    wq_p = wq.reshape(D, QH, 32, 2).transpose(0, 1, 3, 2).reshape(D, QH * HD)
    wk_p = wk.reshape(D, KVH, 32, 2).transpose(0, 1, 3, 2).reshape(D, KVH * HD)
    cosT = np.ascontiguousarray(cos.T)  # [32, T]
    sinT = np.ascontiguousarray(sin.T)
    ctile = np.concatenate([cosT] * 4, axis=0).astype(bf)
    stile = np.concatenate([sinT, -sinT, sinT, -sinT], axis=0).astype(bf)
    wo_t = _tile_k(wo).astype(bf)
    in_maps = []
    for c in range(N_CORES):
        wq_s = _tile_k(wq_p[:, c * 256:(c + 1) * 256]).astype(bf)
        wkv_s = _tile_k(np.concatenate(
            [wk_p[:, c * 64:(c + 1) * 64], wv[:, c * 64:(c + 1) * 64]],
            axis=1)).astype(bf)
        in_maps.append({
            "xT": xt, "wq": wq_s, "wkv": wkv_s, "wo": wo_t,
            "ctile": ctile, "stile": stile,
        })
    return in_maps


def _run(inputs, trace=False):
    if "nc" not in _CACHE:
        _CACHE["nc"] = _build()
    nc = _CACHE["nc"]
    in_maps = _prep_inputs(
        np.asarray(inputs["x"], dtype=np.float32),
        np.asarray(inputs["wq"], dtype=np.float32),
        np.asarray(inputs["wk"], dtype=np.float32),
        np.asarray(inputs["wv"], dtype=np.float32),
        np.asarray(inputs["wo"], dtype=np.float32),
        np.asarray(inputs["cos"], dtype=np.float32),
        np.asarray(inputs["sin"], dtype=np.float32),
    )
    res = run_bass_kernel_spmd(nc, in_maps, core_ids=list(range(N_CORES)),
                               trace=trace)
    full = np.empty((NT, D), dtype=np.float32)
    for c in range(N_CORES):
        o = res.results[c]["out"]
        for b in range(B):
            g = (b * 8 + c) * 128
            full[g:g + 128] = o[b * 128:(b + 1) * 128]
    return full.reshape(B, T, D), res


def kernel(**inputs) -> np.ndarray:
    out, _ = _run(inputs, trace=False)
    return out


def kernel_traced(**inputs):
    out, res = _run(inputs, trace=True)
    return out, res


# revision 25
# speedup vs baseline: 1.1383x; 1.1383x over previous
"""GQA attention forward (B=4, T=1024, D=2048, 32 q-heads / 8 kv-heads, RoPE,
causal) distributed over 8 TRN2 NeuronCores.

Sharding: head-parallel tensor parallelism. Core c owns q-heads 4c..4c+3 and
kv-head c (wq/wk/wv column shards). Attention output (still sharded by head,
transposed layout [head_dim, tokens]) is re-sharded to token-parallel via one
AllToAll (512KB/rank/batch, bf16); each core then computes its 512-token row
slice of the output projection against the full wo.

Device layouts (per core):
  xT    [2048, 4096]  bf16  - x transposed, tokens batch-major
  qT    [128, 4096]x2 bf16  - 2 heads per tile, RoPE'd; head-dim de-interleaved
  kT2   [128, 4096]   bf16  - kv-head kT duplicated in both 64-partition halves
  vT    [64, 4096]    bf16  - PE-transposed per 128-token tile into
                              v[128, 66] with ones columns 64,65 (softmax
                              denominator lands in po rows 64/65 per h01)
  scores sT[k, q] in PSUM -> exp on ACT (scale=1/8 folded) -> bf16
  causal diag mask via gpsimd affine_select (q - k >= 0)
  attn@v: lhsT = v_aug [128, 65+h01], rhs = expT -> psum [65+h01, 512]
  softmax divide: den rows DVE-copied (32-aligned shuffle) into a small pack,
  reciprocal_approx_fast (2 lanes/hp), gpsimd partition_broadcast, fused
  (po x rec) multiply straight into per-head aoT2 tiles
  A2A -> aoT_g [2048(c), 512(t)] -> out[t, e] = sum_c aoT_g[c, t] * wo[c, e]

RoPE with de-interleaved head dims ([32 reals; 32 imags] per 64-row head):
  out = x*C + shift32(x*S), C = [c;c;...], S = [s;-s;s;-s] (host-built tiles);
  the 32-row shift copies run on gpsimd to keep DVE free.
"""

import sys

if "/opt/trn_rl_repo" not in sys.path:
    sys.path.insert(0, "/opt/trn_rl_repo")

import numpy as np
import ml_dtypes

import concourse.bass as bass
import concourse.mybir as mybir
import concourse.tile as tile
from concourse import bacc
from concourse.bass_utils import run_bass_kernel_spmd
from concourse.masks import make_identity

BF16 = mybir.dt.bfloat16
F32 = mybir.dt.float32

B, T, D = 4, 1024, 2048
QH, KVH, HD = 32, 8, 64
N_CORES = 8
NT = B * T            # 4096 global tokens
NKO = D // 128        # 16 contraction subtiles
ROWS = NT // N_CORES  # 512 output rows per core
HPC = QH // N_CORES   # 4 q heads per core

_CACHE = {}


def _enable_ldw_opt():
    # walrus ships with --enable-ldw-opt=false hardcoded; LDWEIGHTS then
    # serializes with matmul streams. Flipping it on FAILS codegen in this
    # walrus build ("InstLdweights is not compatible with LDW optimization"
    # even for plain [128,128] bf16 weights), so it stays opt-in via env.
    import os
    if not os.environ.get("FORCE_LDW_ON"):
        return
    import concourse.bass_utils as _bu
    if getattr(_bu, "_ldw_patched", False):
        return
    _orig = _bu.run_command

    def _patched(argv, **kw):
        argv = [a.replace("--enable-ldw-opt=false", "--enable-ldw-opt=true")
                if isinstance(a, str) else a for a in argv]
        return _orig(argv, **kw)

    _bu.run_command = _patched
    _bu._ldw_patched = True


def _build():
    _enable_ldw_opt()
    nc = bacc.Bacc("TRN2", target_bir_lowering=False, debug=False,
                   num_devices=N_CORES)

    xT = nc.dram_tensor("xT", [8, 128, NKO, 512], BF16, kind="ExternalInput")
    wq = nc.dram_tensor("wq", [128, NKO, HPC * HD], BF16,
                        kind="ExternalInput")
    wkv = nc.dram_tensor("wkv", [128, NKO, 2 * HD], BF16,
                         kind="ExternalInput")
    wo = nc.dram_tensor("wo", [128, NKO, D], BF16, kind="ExternalInput")
    ct = nc.dram_tensor("ctile", [128, T], BF16, kind="ExternalInput")
    st = nc.dram_tensor("stile", [128, T], BF16, kind="ExternalInput")
    out = nc.dram_tensor("out", [ROWS, D], F32, kind="ExternalOutput")

    xT_r = xT.ap()
    wq_r = wq.ap()
    wkv_r = wkv.ap()
    wo_r = wo.ap()

    import contextlib
    with tile.TileContext(nc) as tc, contextlib.ExitStack() as ctx:
        const = ctx.enter_context(tc.tile_pool(name="const", bufs=1))
        xp = ctx.enter_context(tc.tile_pool(name="xp", bufs=2))
        big = ctx.enter_context(tc.tile_pool(name="big", bufs=1))
        vp = ctx.enter_context(tc.tile_pool(name="vp", bufs=2))
        ep = ctx.enter_context(tc.tile_pool(name="ep", bufs=3))
        xsp = ctx.enter_context(tc.tile_pool(name="xsp", bufs=2))
        dnp = ctx.enter_context(tc.tile_pool(name="dnp", bufs=2))
        bcp = ctx.enter_context(tc.tile_pool(name="bcp", bufs=2))
        gp = ctx.enter_context(tc.tile_pool(name="gp", bufs=1))
        op = ctx.enter_context(tc.tile_pool(name="op", bufs=2))
        dram = ctx.enter_context(tc.tile_pool(name="dram", bufs=1,
                                              space="DRAM"))
        pp = ctx.enter_context(tc.tile_pool(name="pp", bufs=2, space="PSUM"))
        sp = ctx.enter_context(tc.tile_pool(name="sp", bufs=2, space="PSUM"))
        ap = ctx.enter_context(tc.tile_pool(name="ap", bufs=2, space="PSUM"))

        # constants / weights (spread across DMA queues so batch-0 x loads
        # aren't stuck behind them on the sync ring)
        wq_sb = const.tile([128, NKO, HPC * HD], BF16, tag="wq")
        nc.scalar.dma_start(wq_sb[:], wq_r)
        wkv_sb = const.tile([128, NKO, 2 * HD], BF16, tag="wkv")
        nc.scalar.dma_start(wkv_sb[:], wkv_r)
        ct_sb = const.tile([128, T], BF16, tag="ct")
        nc.scalar.dma_start(ct_sb[:], ct.ap())
        st_sb = const.tile([128, T], BF16, tag="st")
        nc.scalar.dma_start(st_sb[:], st.ap())
        ident = const.tile([64, 64], BF16, tag="ident")
        make_identity(nc, ident[:])
        # K=1 zero operands: a dummy matmul made from these zeroes a psum
        # bank AND sets every has_written bit, so col-split accumulation
        # below is order-independent
        zrow = const.tile([1, 128], BF16, tag="zrow")
        nc.any.memset(zrow[:], 0.0)
        rrow = const.tile([1, 512], BF16, tag="rrow")
        nc.any.memset(rrow[:], 0.0)

        qT = [big.tile([128, NT], BF16, tag=f"qT{hp}", name=f"qT{hp}")
              for hp in range(2)]
        kT2 = big.tile([128, NT], BF16, tag="kT2")
        # per-head attention output tiles: [64, NT] each so the softmax
        # divide writes land partition-aligned with psum rows 0..63
        aoT2 = [[big.tile([64, NT], BF16, tag=f"aoT{hp}{h01}",
                          name=f"aoT{hp}{h01}") for h01 in range(2)]
                for hp in range(2)]

        in_b = [dram.tile([N_CORES, 256, 128], BF16, tag=f"a2a_in{bb}",
                          name=f"a2a_in{bb}") for bb in range(B)]
        out_b = [dram.tile([N_CORES, 256, 128], BF16, tag=f"a2a_out{bb}",
                           name=f"a2a_out{bb}") for bb in range(B)]
        # full wo stays resident; streamed in during batch-0 compute
        wo_sb = const.tile([128, NKO, D], BF16, tag="wo")
        for n in range(4):
            nc.gpsimd.dma_start(wo_sb[:, :, n * 512:n * 512 + 512],
                                wo_r[:, :, n * 512:n * 512 + 512])

        import os as _os
        _ksplit_on = bool(_os.environ.get("KSPLIT"))

        def ksplit_mms(ps, lhs_of, rhs_of, n_ko):
            # contraction over n_ko K=128 subtiles. Col-split variant: each
            # K=128 matmul becomes two concurrent M=64 matmuls in disjoint
            # PE column groups writing disjoint partition halves of the
            # bank (no per-element write overlap). Only the very first
            # matmul carries start=True (bank-wide has_written clear at its
            # issue); the other column group's ko=0 matmul overwrites its
            # half where bits are unset.
            if not _ksplit_on:
                for ko in range(n_ko):
                    nc.tensor.matmul(
                        ps[0:128, :], lhs_of(ko, 0, 128), rhs_of(ko),
                        start=(ko == 0), stop=(ko == n_ko - 1))
                return
            nc.tensor.matmul(ps[0:128, :], zrow[:], rrow[:],
                             start=True, stop=False, skip_group_check=True)
            for ko in range(n_ko):
                for h in range(2):
                    nc.tensor.matmul(
                        ps[h * 64:h * 64 + 64, :],
                        lhs_of(ko, h * 64, 64), rhs_of(ko),
                        start=False,
                        stop=(ko == n_ko - 1 and h == 1),
                        skip_group_check=True)

        def wo_block(bb):
            # gather the A2A output of batch bb and project my 128 tokens
            aog_b = gp.tile([128, NKO, 128], BF16, tag="aog", bufs=2,
                            name="aog_b")
            nc.scalar.dma_start(
                aog_b[:], out_b[bb].rearrange("s (h p) q -> p (s h) q",
                                              p=128))
            for n in range(4):
                ps = pp.tile([128, 512], F32, tag="mm")
                for kk in range(2 * N_CORES):
                    nc.tensor.matmul(
                        ps[:], aog_b[:, kk, :],
                        wo_sb[:, kk, n * 512:n * 512 + 512],
                        start=(kk == 0), stop=(kk == 2 * N_CORES - 1))
                ot = op.tile([128, 512], F32, tag="ot", bufs=2)
                nc.vector.tensor_copy(ot[:], ps[:])
                nc.scalar.dma_start(
                    out.ap()[bb * 128:bb * 128 + 128,
                             n * 512:n * 512 + 512], ot[:])

        def rope(dst, xs, xs2, ps, rows, cs_sl, ss_sl):
            # dst = ps*C + shift32(ps*S) over `rows` partitions (64 or 128)
            nc.vector.scalar_tensor_tensor(
                dst, ps[0:rows], 1.0, cs_sl[0:rows],
                mybir.AluOpType.mult, mybir.AluOpType.mult)
            nc.vector.scalar_tensor_tensor(
                xs[0:rows], ps[0:rows], 1.0, ss_sl[0:rows],
                mybir.AluOpType.mult, mybir.AluOpType.mult)
            # shift-by-32 within each 64-row half (cross-partition copies)
            for g in range(rows // 32):
                a, b_ = g * 32, (g ^ 1) * 32
                nc.vector.tensor_copy(xs2[a:a + 32], xs[b_:b_ + 32])
            nc.vector.tensor_add(dst, dst, xs2[0:rows])

        for b in range(B):
            xc = []
            for half in range(2):
                xt = xp.tile([128, NKO, 512], BF16, tag="x")
                eng = nc.sync if half == 0 else nc.scalar
                for kq in range(4):
                    eng.dma_start(
                        xt[:, kq * 4:kq * 4 + 4, :],
                        xT_r[b * 2 + half, :, kq * 4:kq * 4 + 4])
                xc.append(xt)

            # q projection + rope
            for hp in range(2):
                for half in range(2):
                    t0 = b * T + half * 512
                    ps = pp.tile([128, 512], F32, tag="mm")
                    for ko in range(NKO):
                        nc.tensor.matmul(
                            ps[:], wq_sb[:, ko, hp * 128:hp * 128 + 128],
                            xc[half][:, ko, :],
                            start=(ko == 0), stop=(ko == NKO - 1))
                    xs = xsp.tile([128, 512], BF16, tag="xs")
                    xs2 = xsp.tile([128, 512], BF16, tag="xs2")
                    csl = ct_sb[:, half * 512:half * 512 + 512]
                    ssl = st_sb[:, half * 512:half * 512 + 512]
                    rope(qT[hp][:, t0:t0 + 512], xs, xs2, ps, 128, csl, ssl)

            # kv projection: rope k into both halves of kT2
            vstages = []
            for half in range(2):
                t0 = b * T + half * 512
                ps = pp.tile([128, 512], F32, tag="mm")
                for ko in range(NKO):
                    nc.tensor.matmul(
                        ps[:], wkv_sb[:, ko, :], xc[half][:, ko, :],
                        start=(ko == 0), stop=(ko == NKO - 1))
                xs = xsp.tile([128, 512], BF16, tag="xs")
                xs2 = xsp.tile([128, 512], BF16, tag="xs2")
                csl = ct_sb[:, half * 512:half * 512 + 512]
                ssl = st_sb[:, half * 512:half * 512 + 512]
                rope(kT2[0:64, t0:t0 + 512], xs, xs2, ps, 64, csl, ssl)
                nc.vector.tensor_copy(kT2[64:128, t0:t0 + 512],
                                      kT2[0:64, t0:t0 + 512])
                vstage = xsp.tile([64, 512], BF16, tag="vstage",
                                  name=f"vstage{half}")
                nc.vector.tensor_copy(vstage[:], ps[64:128, :])
                vstages.append(vstage)

            # v into natural layout [tok, 64] + a ones column (softmax
            # denominator trick: den lands in po row 64)
            v_b = vp.tile([128, 8, 66], BF16, tag="v")
            for j in range(8):
                tps = pp.tile([128, 64], BF16, tag="mm")
                nc.tensor.transpose(
                    tps[:],
                    vstages[j // 4][:, (j % 4) * 128:(j % 4) * 128 + 128],
                    ident[:])
                nc.vector.tensor_copy(v_b[:, j, 0:HD], tps[:])
            nc.any.memset(v_b[:, :, HD:66], 1.0)

            # attention: all 4 heads advance together per (chunk, k-tile) so
            # the PE stream stays dense; K=64 score matmuls for h01=0/1 sit
            # in disjoint row groups and overlap in the array
            in_r = in_b[b].rearrange("j r q -> r j q")
            for c in range(2):
                for hp in range(2):
                    po2 = [ap.tile([65, 512], F32, tag="attn",
                                   name=f"po{h01}") for h01 in range(2)]
                    nj = 4 * c + 4
                    for j in range(nj):
                        q_lo = max(c * 512, j * 128)
                        N = (c + 1) * 512 - q_lo
                        q_rel = q_lo - c * 512
                        ks = b * T + j * 128
                        # both heads' scores into one 2-bank psum tile so
                        # exp runs once over [128, 2, N]
                        ps = sp.tile([128, 2, 512], F32, tag="score")
                        et = ep.tile([128, 2, 512], BF16, tag="et")
                        for h01 in range(2):
                            hbase = h01 * 64
                            nc.tensor.matmul(
                                ps[:, h01, :N],
                                kT2[hbase:hbase + 64, ks:ks + 128],
                                qT[hp][hbase:hbase + 64,
                                       b * T + q_lo:b * T + q_lo + N],
                                start=True, stop=True)
                        nc.scalar.activation(
                            et[:, :, :N], ps[:, :, :N],
                            mybir.ActivationFunctionType.Exp, scale=0.125)
                        if j >= 4 * c:
                            # causal diagonal block: keep q >= k, zero rest
                            # (2D per h01, the shape affine_select is proven
                            # on in masks.py)
                            for h01 in range(2):
                                nc.gpsimd.affine_select(
                                    out=et[:, h01, 0:128],
                                    in_=et[:, h01, 0:128],
                                    compare_op=mybir.AluOpType.is_ge,
                                    fill=0.0, base=0,
                                    pattern=[[1, 128]],
                                    channel_multiplier=-1)
                        for h01 in range(2):
                            nc.tensor.matmul(
                                po2[h01][0:65, q_rel:512],
                                v_b[:, j, 0:65],
                                et[:, h01, :N],
                                start=(j == 0), stop=(j == nj - 1))
                    # softmax normalization for this hp's 2 heads: fast
                    # reciprocal straight off the psum den row (aligned
                    # 64->0 shuffle), broadcast, then a fused (po * rec)
                    # multiply into the per-head aoT2 tile
                    g0 = (b * 2 + c) * 512
                    for h01 in range(2):
                        den1 = dnp.tile([1, 512], F32, tag="den", bufs=2)
                        nc.scalar.copy(den1[:], po2[h01][64:65, :])
                        rec1 = dnp.tile([1, 512], F32, tag="rec", bufs=2)
                        nc.vector.reciprocal_approx_fast(rec1[:], den1[:])
                        bc = bcp.tile([64, 512], F32, tag="bc", bufs=2)
                        nc.gpsimd.partition_broadcast(bc[:], rec1[:])
                        nc.vector.tensor_mul(
                            aoT2[hp][h01][:, g0:g0 + 512],
                            po2[h01][0:64, :], bc[:])
                    # stream this chunk's A2A input blocks out once divided:
                    # batch-b slot j carries my rows for token block b*8+j
                    for h01 in range(2):
                        nc.sync.dma_start(
                            in_r[hp * 128 + h01 * 64:
                                 hp * 128 + h01 * 64 + 64,
                                 c * 4:c * 4 + 4, :],
                            aoT2[hp][h01][:, g0:g0 + 512].rearrange(
                                "p (j q) -> p j q", j=4))

            # fire this batch's re-shard; its consumption (gather + wo) is
            # deferred by one batch so nothing ever waits on an in-flight
            # collective
            nc.gpsimd.collective_compute(
                "AllToAll", mybir.AluOpType.bypass,
                replica_groups=[list(range(N_CORES))],
                ins=[in_b[b].opt()], outs=[out_b[b].opt()])

            # consume A2As two batches late so the in-order PE stream never
            # parks on an in-flight collective (a full batch of compute
            # covers even a skew-delayed collective)
            if b >= 2:
                wo_block(b - 2)

        wo_block(B - 2)
        wo_block(B - 1)

    nc.compile()
    return nc


def _tile_k(w):
    # [D, M] -> [128, D//128, M] with d = ko*128 + p, contiguous per partition
    return np.ascontiguousarray(
        w.reshape(NKO, 128, w.shape[1]).transpose(1, 0, 2))


def _prep_inputs(x, wq, wk, wv, wo, cos, sin):
    bf = ml_dtypes.bfloat16
    x2 = x.reshape(NT, D).T  # [D, NT]
    # [8 chunks, 128, NKO, 512]: one contiguous 16KB run per partition
    xt = np.ascontiguousarray(
        x2.reshape(NKO, 128, 8, 512).transpose(2, 1, 0, 3)).astype(bf)
    # de-interleave rope pairs within each head: col j -> (j%2)*32 + j//2
    wq_p = wq.reshape(D, QH, 32, 2).transpose(0, 1, 3, 2).reshape(D, QH * HD)
    wk_p = wk.reshape(D, KVH, 32, 2).transpose(0, 1, 3, 2).reshape(D, KVH * HD)
    cosT = np.ascontiguousarray(cos.T)  # [32, T]
    sinT = np.ascontiguousarray(sin.T)
    ctile = np.concatenate([cosT] * 4, axis=0).astype(bf)
    stile = np.concatenate([sinT, -sinT, sinT, -sinT], axis=0).astype(bf)
    wo_t = _tile_k(wo).astype(bf)
    in_maps = []
    for c in range(N_CORES):
        wq_s = _tile_k(wq_p[:, c * 256:(c + 1) * 256]).astype(bf)
        wkv_s = _tile_k(np.concatenate(
            [wk_p[:, c * 64:(c + 1) * 64], wv[:, c * 64:(c + 1) * 64]],
            axis=1)).astype(bf)
        in_maps.append({
            "xT": xt, "wq": wq_s, "wkv": wkv_s, "wo": wo_t,
            "ctile": ctile, "stile": stile,
        })
    return in_maps


def _run(inputs, trace=False):
    if "nc" not in _CACHE:
        _CACHE["nc"] = _build()
    nc = _CACHE["nc"]
    in_maps = _prep_inputs(
        np.asarray(inputs["x"], dtype=np.float32),
        np.asarray(inputs["wq"], dtype=np.float32),
        np.asarray(inputs["wk"], dtype=np.float32),
        np.asarray(inputs["wv"], dtype=np.float32),
        np.asarray(inputs["wo"], dtype=np.float32),
        np.asarray(inputs["cos"], dtype=np.float32),
        np.asarray(inputs["sin"], dtype=np.float32),
    )
    res = run_bass_kernel_spmd(nc, in_maps, core_ids=list(range(N_CORES)),
                               trace=trace)
    full = np.empty((NT, D), dtype=np.float32)
    for c in range(N_CORES):
        o = res.results[c]["out"]
        for b in range(B):
            g = (b * 8 + c) * 128
            full[g:g + 128] = o[b * 128:(b + 1) * 128]
    return full.reshape(B, T, D), res


def kernel(**inputs) -> np.ndarray:
    out, _ = _run(inputs, trace=False)
    return out


def kernel_traced(**inputs):
    out, res = _run(inputs, trace=True)
    return out, res


# revision 28
# speedup vs baseline: 1.1887x; 1.0443x over previous
"""GQA attention forward (B=4, T=1024, D=2048, 32 q-heads / 8 kv-heads, RoPE,
causal) distributed over 8 TRN2 NeuronCores.

Sharding: head-parallel tensor parallelism. Core c owns q-heads 4c..4c+3 and
kv-head c (wq/wk/wv column shards). Attention output (still sharded by head,
transposed layout [head_dim, tokens]) is re-sharded to token-parallel via one
AllToAll (512KB/rank/batch, bf16); each core then computes its 512-token row
slice of the output projection against the full wo.

Device layouts (per core):
  xT    [2048, 4096]  bf16  - x transposed, tokens batch-major
  qT    [128, 4096]x2 bf16  - 2 heads per tile, RoPE'd; head-dim de-interleaved
  kT2   [128, 4096]   bf16  - kv-head kT duplicated in both 64-partition halves
  vT    [64, 4096]    bf16  - PE-transposed per 128-token tile into
                              v[128, 66] with ones columns 64,65 (softmax
                              denominator lands in po rows 64/65 per h01)
  scores sT[k, q] in PSUM -> exp on ACT (scale=1/8 folded) -> bf16
  causal diag mask via gpsimd affine_select (q - k >= 0), one 2D call per h01
  attn@v: lhsT = v_aug [128, 65], rhs = expT -> psum po [65, 512]
  softmax divide: den row po[64] ACT-copied to SBUF (custom-DVE ops misread
  PSUM on HW!), reciprocal_approx_fast, gpsimd partition_broadcast, then one
  fused (po x rec) multiply straight into the per-head aoT2 tile
  A2A (fired per batch, consumed TWO batches later so the in-order PE queue
  never parks on an in-flight collective) -> aoT_g [2048(c), 512(t)] ->
  out[t, e] = sum_c aoT_g[c, t] * wo[c, e]

RoPE with de-interleaved head dims ([32 reals; 32 imags] per 64-row head):
  out = x*C + shift32(x*S), C = [c;c;...], S = [s;-s;s;-s] (host-built tiles).

Known dead ends on this toolchain (do not retry blindly): walrus
--enable-ldw-opt=true fails codegen on any bass LDWEIGHTS; row-group K-split
concurrent same-bank accumulation crashes the device; col-split M=64 pairs
and weight-reuse matmul pairing are both correct but SLOWER (walrus emits an
LDWEIGHTS per matmul regardless, ~463ns/N=512-matmul is the compiler floor).
"""

import sys

if "/opt/trn_rl_repo" not in sys.path:
    sys.path.insert(0, "/opt/trn_rl_repo")

import numpy as np
import ml_dtypes

import concourse.bass as bass
import concourse.mybir as mybir
import concourse.tile as tile
from concourse import bacc
from concourse.bass_utils import run_bass_kernel_spmd
from concourse.masks import make_identity

BF16 = mybir.dt.bfloat16
F32 = mybir.dt.float32

B, T, D = 4, 1024, 2048
QH, KVH, HD = 32, 8, 64
N_CORES = 8
NT = B * T            # 4096 global tokens
NKO = D // 128        # 16 contraction subtiles
ROWS = NT // N_CORES  # 512 output rows per core
HPC = QH // N_CORES   # 4 q heads per core

_CACHE = {}


def _enable_ldw_opt():
    # walrus ships with --enable-ldw-opt=false hardcoded; LDWEIGHTS then
    # serializes with matmul streams. Flipping it on FAILS codegen in this
    # walrus build ("InstLdweights is not compatible with LDW optimization"
    # even for plain [128,128] bf16 weights), so it stays opt-in via env.
    import os
    if not os.environ.get("FORCE_LDW_ON"):
        return
    import concourse.bass_utils as _bu
    if getattr(_bu, "_ldw_patched", False):
        return
    _orig = _bu.run_command

    def _patched(argv, **kw):
        argv = [a.replace("--enable-ldw-opt=false", "--enable-ldw-opt=true")
                if isinstance(a, str) else a for a in argv]
        return _orig(argv, **kw)

    _bu.run_command = _patched
    _bu._ldw_patched = True


def _build():
    _enable_ldw_opt()
    nc = bacc.Bacc("TRN2", target_bir_lowering=False, debug=False,
                   num_devices=N_CORES)

    xT = nc.dram_tensor("xT", [8, 128, NKO, 512], BF16, kind="ExternalInput")
    wq = nc.dram_tensor("wq", [128, NKO, HPC * HD], BF16,
                        kind="ExternalInput")
    wkv = nc.dram_tensor("wkv", [128, NKO, 2 * HD], BF16,
                         kind="ExternalInput")
    wo = nc.dram_tensor("wo", [128, NKO, D], BF16, kind="ExternalInput")
    ct = nc.dram_tensor("ctile", [128, T], BF16, kind="ExternalInput")
    st = nc.dram_tensor("stile", [128, T], BF16, kind="ExternalInput")
    out = nc.dram_tensor("out", [ROWS, D], F32, kind="ExternalOutput")

    xT_r = xT.ap()
    wq_r = wq.ap()
    wkv_r = wkv.ap()
    wo_r = wo.ap()

    import contextlib
    with tile.TileContext(nc) as tc, contextlib.ExitStack() as ctx:
        const = ctx.enter_context(tc.tile_pool(name="const", bufs=1))
        xp = ctx.enter_context(tc.tile_pool(name="xp", bufs=2))
        big = ctx.enter_context(tc.tile_pool(name="big", bufs=1))
        vp = ctx.enter_context(tc.tile_pool(name="vp", bufs=2))
        ep = ctx.enter_context(tc.tile_pool(name="ep", bufs=3))
        xsp = ctx.enter_context(tc.tile_pool(name="xsp", bufs=2))
        dnp = ctx.enter_context(tc.tile_pool(name="dnp", bufs=2))
        bcp = ctx.enter_context(tc.tile_pool(name="bcp", bufs=2))
        gp = ctx.enter_context(tc.tile_pool(name="gp", bufs=1))
        op = ctx.enter_context(tc.tile_pool(name="op", bufs=2))
        dram = ctx.enter_context(tc.tile_pool(name="dram", bufs=1,
                                              space="DRAM"))
        pp = ctx.enter_context(tc.tile_pool(name="pp", bufs=2, space="PSUM"))
        sp = ctx.enter_context(tc.tile_pool(name="sp", bufs=2, space="PSUM"))
        ap = ctx.enter_context(tc.tile_pool(name="ap", bufs=2, space="PSUM"))

        # constants / weights (spread across DMA queues so batch-0 x loads
        # aren't stuck behind them on the sync ring)
        wq_sb = const.tile([128, NKO, HPC * HD], BF16, tag="wq")
        nc.scalar.dma_start(wq_sb[:], wq_r)
        wkv_sb = const.tile([128, NKO, 2 * HD], BF16, tag="wkv")
        nc.scalar.dma_start(wkv_sb[:], wkv_r)
        ct_sb = const.tile([128, T], BF16, tag="ct")
        nc.scalar.dma_start(ct_sb[:], ct.ap())
        st_sb = const.tile([128, T], BF16, tag="st")
        nc.scalar.dma_start(st_sb[:], st.ap())
        ident = const.tile([64, 64], BF16, tag="ident")
        make_identity(nc, ident[:])
        # K=1 zero operands: a dummy matmul made from these zeroes a psum
        # bank AND sets every has_written bit, so col-split accumulation
        # below is order-independent
        zrow = const.tile([1, 128], BF16, tag="zrow")
        nc.any.memset(zrow[:], 0.0)
        rrow = const.tile([1, 512], BF16, tag="rrow")
        nc.any.memset(rrow[:], 0.0)

        qT = [big.tile([128, NT], BF16, tag=f"qT{hp}", name=f"qT{hp}")
              for hp in range(2)]
        kT2 = big.tile([128, NT], BF16, tag="kT2")
        # per-head attention output tiles: [64, NT] each so the softmax
        # divide writes land partition-aligned with psum rows 0..63
        aoT2 = [[big.tile([64, NT], BF16, tag=f"aoT{hp}{h01}",
                          name=f"aoT{hp}{h01}") for h01 in range(2)]
                for hp in range(2)]

        in_b = [dram.tile([N_CORES, 256, 128], BF16, tag=f"a2a_in{bb}",
                          name=f"a2a_in{bb}") for bb in range(B)]
        out_b = [dram.tile([N_CORES, 256, 128], BF16, tag=f"a2a_out{bb}",
                           name=f"a2a_out{bb}") for bb in range(B)]
        # full wo stays resident; streamed in during batch-0 compute
        wo_sb = const.tile([128, NKO, D], BF16, tag="wo")
        for n in range(4):
            nc.gpsimd.dma_start(wo_sb[:, :, n * 512:n * 512 + 512],
                                wo_r[:, :, n * 512:n * 512 + 512])

        import os as _os
        _ksplit_on = bool(_os.environ.get("KSPLIT"))

        def ksplit_mms(ps, lhs_of, rhs_of, n_ko):
            # contraction over n_ko K=128 subtiles. Col-split variant: each
            # K=128 matmul becomes two concurrent M=64 matmuls in disjoint
            # PE column groups writing disjoint partition halves of the
            # bank (no per-element write overlap). Only the very first
            # matmul carries start=True (bank-wide has_written clear at its
            # issue); the other column group's ko=0 matmul overwrites its
            # half where bits are unset.
            if not _ksplit_on:
                for ko in range(n_ko):
                    nc.tensor.matmul(
                        ps[0:128, :], lhs_of(ko, 0, 128), rhs_of(ko),
                        start=(ko == 0), stop=(ko == n_ko - 1))
                return
            nc.tensor.matmul(ps[0:128, :], zrow[:], rrow[:],
                             start=True, stop=False, skip_group_check=True)
            for ko in range(n_ko):
                for h in range(2):
                    nc.tensor.matmul(
                        ps[h * 64:h * 64 + 64, :],
                        lhs_of(ko, h * 64, 64), rhs_of(ko),
                        start=False,
                        stop=(ko == n_ko - 1 and h == 1),
                        skip_group_check=True)

        def wo_block(bb):
            # gather the A2A output of batch bb and project my 128 tokens
            aog_b = gp.tile([128, NKO, 128], BF16, tag="aog", bufs=2,
                            name="aog_b")
            nc.scalar.dma_start(
                aog_b[:], out_b[bb].rearrange("s (h p) q -> p (s h) q",
                                              p=128))
            for n in range(4):
                ps = pp.tile([128, 512], F32, tag="mm")
                for kk in range(2 * N_CORES):
                    nc.tensor.matmul(
                        ps[:], aog_b[:, kk, :],
                        wo_sb[:, kk, n * 512:n * 512 + 512],
                        start=(kk == 0), stop=(kk == 2 * N_CORES - 1))
                ot = op.tile([128, 512], F32, tag="ot", bufs=2)
                nc.vector.tensor_copy(ot[:], ps[:])
                nc.scalar.dma_start(
                    out.ap()[bb * 128:bb * 128 + 128,
                             n * 512:n * 512 + 512], ot[:])

        def rope(dst, xs, xs2, ps, rows, cs_sl, ss_sl):
            # dst = ps*C + shift32(ps*S) over `rows` partitions (64 or 128)
            nc.vector.scalar_tensor_tensor(
                dst, ps[0:rows], 1.0, cs_sl[0:rows],
                mybir.AluOpType.mult, mybir.AluOpType.mult)
            nc.vector.scalar_tensor_tensor(
                xs[0:rows], ps[0:rows], 1.0, ss_sl[0:rows],
                mybir.AluOpType.mult, mybir.AluOpType.mult)
            # shift-by-32 within each 64-row half (cross-partition copies)
            for g in range(rows // 32):
                a, b_ = g * 32, (g ^ 1) * 32
                nc.vector.tensor_copy(xs2[a:a + 32], xs[b_:b_ + 32])
            nc.vector.tensor_add(dst, dst, xs2[0:rows])

        for b in range(B):
            xc = []
            for half in range(2):
                xt = xp.tile([128, NKO, 512], BF16, tag="x")
                eng = nc.sync if half == 0 else nc.scalar
                for kq in range(4):
                    eng.dma_start(
                        xt[:, kq * 4:kq * 4 + 4, :],
                        xT_r[b * 2 + half, :, kq * 4:kq * 4 + 4])
                xc.append(xt)

            # q projection + rope
            for hp in range(2):
                for half in range(2):
                    t0 = b * T + half * 512
                    ps = pp.tile([128, 512], F32, tag="mm")
                    for ko in range(NKO):
                        nc.tensor.matmul(
                            ps[:], wq_sb[:, ko, hp * 128:hp * 128 + 128],
                            xc[half][:, ko, :],
                            start=(ko == 0), stop=(ko == NKO - 1))
                    xs = xsp.tile([128, 512], BF16, tag="xs")
                    xs2 = xsp.tile([128, 512], BF16, tag="xs2")
                    csl = ct_sb[:, half * 512:half * 512 + 512]
                    ssl = st_sb[:, half * 512:half * 512 + 512]
                    rope(qT[hp][:, t0:t0 + 512], xs, xs2, ps, 128, csl, ssl)

            # kv projection: rope k into both halves of kT2
            vstages = []
            for half in range(2):
                t0 = b * T + half * 512
                ps = pp.tile([128, 512], F32, tag="mm")
                for ko in range(NKO):
                    nc.tensor.matmul(
                        ps[:], wkv_sb[:, ko, :], xc[half][:, ko, :],
                        start=(ko == 0), stop=(ko == NKO - 1))
                xs = xsp.tile([128, 512], BF16, tag="xs")
                xs2 = xsp.tile([128, 512], BF16, tag="xs2")
                csl = ct_sb[:, half * 512:half * 512 + 512]
                ssl = st_sb[:, half * 512:half * 512 + 512]
                rope(kT2[0:64, t0:t0 + 512], xs, xs2, ps, 64, csl, ssl)
                nc.vector.tensor_copy(kT2[64:128, t0:t0 + 512],
                                      kT2[0:64, t0:t0 + 512])
                vstage = xsp.tile([64, 512], BF16, tag="vstage",
                                  name=f"vstage{half}")
                nc.vector.tensor_copy(vstage[:], ps[64:128, :])
                vstages.append(vstage)

            # v into natural layout [tok, 64] + a ones column (softmax
            # denominator trick: den lands in po row 64)
            v_b = vp.tile([128, 8, 66], BF16, tag="v")
            for j in range(8):
                tps = pp.tile([128, 64], BF16, tag="mm")
                nc.tensor.transpose(
                    tps[:],
                    vstages[j // 4][:, (j % 4) * 128:(j % 4) * 128 + 128],
                    ident[:])
                nc.vector.tensor_copy(v_b[:, j, 0:HD], tps[:])
            nc.any.memset(v_b[:, :, HD:66], 1.0)

            # attention: all 4 heads advance together per (chunk, k-tile) so
            # the PE stream stays dense; K=64 score matmuls for h01=0/1 sit
            # in disjoint row groups and overlap in the array
            in_r = in_b[b].rearrange("j r q -> r j q")
            for c in range(2):
                for hp in range(2):
                    po2 = [ap.tile([65, 512], F32, tag="attn",
                                   name=f"po{h01}") for h01 in range(2)]
                    nj = 4 * c + 4
                    for j in range(nj):
                        q_lo = max(c * 512, j * 128)
                        N = (c + 1) * 512 - q_lo
                        q_rel = q_lo - c * 512
                        ks = b * T + j * 128
                        # both heads' scores into one 2-bank psum tile so
                        # exp runs once over [128, 2, N]
                        ps = sp.tile([128, 2, 512], F32, tag="score")
                        et = ep.tile([128, 2, 512], BF16, tag="et")
                        for h01 in range(2):
                            hbase = h01 * 64
                            nc.tensor.matmul(
                                ps[:, h01, :N],
                                kT2[hbase:hbase + 64, ks:ks + 128],
                                qT[hp][hbase:hbase + 64,
                                       b * T + q_lo:b * T + q_lo + N],
                                start=True, stop=True)
                        nc.scalar.activation(
                            et[:, :, :N], ps[:, :, :N],
                            mybir.ActivationFunctionType.Exp, scale=0.125)
                        if j >= 4 * c:
                            # causal diagonal block: keep q >= k, zero rest
                            # (2D per h01, the shape affine_select is proven
                            # on in masks.py)
                            for h01 in range(2):
                                nc.gpsimd.affine_select(
                                    out=et[:, h01, 0:128],
                                    in_=et[:, h01, 0:128],
                                    compare_op=mybir.AluOpType.is_ge,
                                    fill=0.0, base=0,
                                    pattern=[[1, 128]],
                                    channel_multiplier=-1)
                        for h01 in range(2):
                            nc.tensor.matmul(
                                po2[h01][0:65, q_rel:512],
                                v_b[:, j, 0:65],
                                et[:, h01, :N],
                                start=(j == 0), stop=(j == nj - 1))
                    # softmax normalization for this hp's 2 heads: fast
                    # reciprocal straight off the psum den row (aligned
                    # 64->0 shuffle), broadcast, then a fused (po * rec)
                    # multiply into the per-head aoT2 tile
                    g0 = (b * 2 + c) * 512
                    for h01 in range(2):
                        den1 = dnp.tile([1, 512], F32, tag="den", bufs=2)
                        nc.scalar.copy(den1[:], po2[h01][64:65, :])
                        rec1 = dnp.tile([1, 512], F32, tag="rec", bufs=2)
                        nc.vector.reciprocal_approx_fast(rec1[:], den1[:])
                        bc = bcp.tile([64, 512], F32, tag="bc", bufs=2)
                        nc.gpsimd.partition_broadcast(bc[:], rec1[:])
                        nc.vector.tensor_mul(
                            aoT2[hp][h01][:, g0:g0 + 512],
                            po2[h01][0:64, :], bc[:])
                    # stream this chunk's A2A input blocks out once divided:
                    # batch-b slot j carries my rows for token block b*8+j
                    for h01 in range(2):
                        nc.sync.dma_start(
                            in_r[hp * 128 + h01 * 64:
                                 hp * 128 + h01 * 64 + 64,
                                 c * 4:c * 4 + 4, :],
                            aoT2[hp][h01][:, g0:g0 + 512].rearrange(
                                "p (j q) -> p j q", j=4))

            # fire this batch's re-shard; its consumption (gather + wo) is
            # deferred by one batch so nothing ever waits on an in-flight
            # collective
            nc.gpsimd.collective_compute(
                "AllToAll", mybir.AluOpType.bypass,
                replica_groups=[list(range(N_CORES))],
                ins=[in_b[b].opt()], outs=[out_b[b].opt()])

            # consume A2As two batches late so the in-order PE stream never
            # parks on an in-flight collective (a full batch of compute
            # covers even a skew-delayed collective)
            if b >= 2:
                wo_block(b - 2)

        wo_block(B - 2)
        wo_block(B - 1)

    nc.compile()
    return nc


def _tile_k(w):
    # [D, M] -> [128, D//128, M] with d = ko*128 + p, contiguous per partition
    return np.ascontiguousarray(
        w.reshape(NKO, 128, w.shape[1]).transpose(1, 0, 2))


def _prep_inputs(x, wq, wk, wv, wo, cos, sin):
    bf = ml_dtypes.bfloat16
    x2 = x.reshape(NT, D).T  # [D, NT]
    # [8 chunks, 128, NKO, 512]: one contiguous 16KB run per partition
    xt = np.ascontiguousarray(
        x2.reshape(NKO, 128, 8, 512).transpose(2, 1, 0, 3)).astype(bf)
    # de-interleave rope pairs within each head: col j -> (j%2)*32 + j//2
    wq_p = wq.reshape(D, QH, 32, 2).transpose(0, 1, 3, 2).reshape(D, QH * HD)
    wk_p = wk.reshape(D, KVH, 32, 2).transpose(0, 1, 3, 2).reshape(D, KVH * HD)
    cosT = np.ascontiguousarray(cos.T)  # [32, T]
    sinT = np.ascontiguousarray(sin.T)
    ctile = np.concatenate([cosT] * 4, axis=0).astype(bf)
    stile = np.concatenate([sinT, -sinT, sinT, -sinT], axis=0).astype(bf)
    wo_t = _tile_k(wo).astype(bf)
    in_maps = []
    for c in range(N_CORES):
        wq_s = _tile_k(wq_p[:, c * 256:(c + 1) * 256]).astype(bf)
        wkv_s = _tile_k(np.concatenate(
            [wk_p[:, c * 64:(c + 1) * 64], wv[:, c * 64:(c + 1) * 64]],
            axis=1)).astype(bf)
        in_maps.append({
            "xT": xt, "wq": wq_s, "wkv": wkv_s, "wo": wo_t,
            "ctile": ctile, "stile": stile,
        })
    return in_maps


def _run(inputs, trace=False):
    if "nc" not in _CACHE:
        _CACHE["nc"] = _build()
    nc = _CACHE["nc"]
    in_maps = _prep_inputs(
        np.asarray(inputs["x"], dtype=np.float32),
        np.asarray(inputs["wq"], dtype=np.float32),
        np.asarray(inputs["wk"], dtype=np.float32),
        np.asarray(inputs["wv"], dtype=np.float32),
        np.asarray(inputs["wo"], dtype=np.float32),
        np.asarray(inputs["cos"], dtype=np.float32),
        np.asarray(inputs["sin"], dtype=np.float32),
    )
    res = run_bass_kernel_spmd(nc, in_maps, core_ids=list(range(N_CORES)),
                               trace=trace)
    full = np.empty((NT, D), dtype=np.float32)
    for c in range(N_CORES):
        o = res.results[c]["out"]
        for b in range(B):
            g = (b * 8 + c) * 128
            full[g:g + 128] = o[b * 128:(b + 1) * 128]
    return full.reshape(B, T, D), res


def kernel(**inputs) -> np.ndarray:
    out, _ = _run(inputs, trace=False)
    return out


def kernel_traced(**inputs):
    out, res = _run(inputs, trace=True)
    return out, res
